# revision 2
# baseline (speedup 1.0000x reference)
"""Trainium2 Bass kernel for y = x @ W.T + b  (x: [16384,1024] f32,
W: [1024,1024] f32, b: [1024] f32) on 8 NeuronCores.

Data-parallel: x is split along batch into 8 shards of 2048 rows;
W and b are replicated. Each core computes its y shard with bf16
matmuls accumulating in fp32 PSUM; bias is fused into the PSUM->SBUF
eviction on the Scalar engine. Host-side we pre-transpose x (and W) to
put the contraction dim on SBUF partitions, so no on-chip transposes
are needed.
"""

import sys

if "/opt/trn_rl_repo" not in sys.path:
    sys.path.insert(0, "/opt/trn_rl_repo")

import ml_dtypes
import numpy as np

BATCH = 16384
IN_F = 1024
OUT_F = 1024
NCORES = 8
P = 128
KO = IN_F // P  # 8 contraction tiles
MO = OUT_F // P  # 8 output-feature tiles
BS = BATCH // NCORES  # 2048 rows per core
FD = 512  # matmul moving free dim (one PSUM bank of fp32)
NB = BS // FD  # 4 moving chunks per output tile

_cache = {}
LAST_RESULT = None


def _build():
    import concourse.mybir as mybir
    import concourse.tile as tile
    from concourse import bacc

    nc = bacc.Bacc(None, target_bir_lowering=False)
    xT = nc.declare_dram_parameter("xT", [P, KO, BS], mybir.dt.bfloat16, isOutput=False)
    wT = nc.declare_dram_parameter(
        "wT", [P, KO, OUT_F], mybir.dt.bfloat16, isOutput=False
    )
    bias = nc.declare_dram_parameter("bias", [P, MO], mybir.dt.float32, isOutput=False)
    out = nc.declare_dram_parameter("out", [P, MO, BS], mybir.dt.float32, isOutput=True)

    with tile.TileContext(nc) as tc:
        with (
            tc.tile_pool(name="const", bufs=1) as cpool,
            tc.tile_pool(name="outp", bufs=3) as opool,
            tc.tile_pool(name="psum", bufs=2, space="PSUM") as ppool,
        ):
            x_sb = cpool.tile([P, KO, BS], mybir.dt.bfloat16)
            w_sb = cpool.tile([P, KO, OUT_F], mybir.dt.bfloat16)
            b_sb = cpool.tile([P, MO], mybir.dt.float32)
            nc.sync.dma_start(b_sb[:], bias[:])
            # Per-ko chunks so the first matmuls can start before the
            # whole 6 MiB of inputs has landed.
            for ko in range(KO):
                nc.sync.dma_start(w_sb[:, ko], wT[:, ko])
                nc.sync.dma_start(x_sb[:, ko], xT[:, ko])

            for mo in range(MO):
                ps = ppool.tile([P, BS], mybir.dt.float32)
                for ko in range(KO):
                    for nb in range(NB):
                        nc.tensor.matmul(
                            ps[:, nb * FD : (nb + 1) * FD],
                            w_sb[:, ko, mo * P : (mo + 1) * P],
                            x_sb[:, ko, nb * FD : (nb + 1) * FD],
                            start=(ko == 0),
                            stop=(ko == KO - 1),
                        )
                o_sb = opool.tile([P, BS], mybir.dt.float32)
                nc.scalar.activation(
                    o_sb[:],
                    ps[:],
                    mybir.ActivationFunctionType.Identity,
                    bias=b_sb[:, mo : mo + 1],
                )
                nc.sync.dma_start(out[:, mo], o_sb[:])

    nc.compile()
    return nc


def kernel(x, weight, bias):
    global LAST_RESULT
    from concourse.bass_utils import run_bass_kernel_spmd

    if "nc" not in _cache:
        _cache["nc"] = _build()
    nc = _cache["nc"]

    bf16 = ml_dtypes.bfloat16
    # W.T laid out [P, KO, OUT_F]: wT[p, ko, o] = W[o, ko*P + p]
    wT = np.ascontiguousarray(
        weight.astype(bf16).T.reshape(KO, P, OUT_F).transpose(1, 0, 2)
    )
    # bias laid out [P, MO]: b[p, mo] = bias[mo*P + p]
    b_t = np.ascontiguousarray(bias.astype(np.float32).reshape(MO, P).T)

    in_maps = []
    for c in range(NCORES):
        xs = x[c * BS : (c + 1) * BS].astype(bf16)
        # x.T laid out [P, KO, BS]: xT[p, ko, b] = x[b, ko*P + p]
        xT = np.ascontiguousarray(xs.T.reshape(KO, P, BS).transpose(1, 0, 2))
        in_maps.append({"xT": xT, "wT": wT, "bias": b_t})

    res = run_bass_kernel_spmd(nc, in_maps, list(range(NCORES)))
    LAST_RESULT = res

    y = np.empty((BATCH, OUT_F), dtype=np.float32)
    for c in range(NCORES):
        o = res.results[c]["out"]  # [P, MO, BS]
        y[c * BS : (c + 1) * BS] = o.transpose(2, 1, 0).reshape(BS, OUT_F)
    return y


# revision 3
# speedup vs baseline: 1.0535x; 1.0535x over previous
"""Trainium2 Bass kernel for y = x @ W.T + b  (x: [16384,1024] f32,
W: [1024,1024] f32, b: [1024] f32) on 8 NeuronCores.

Data-parallel: x is split along batch into 8 shards of 2048 rows;
W and b are replicated. Each core computes its y shard with bf16
matmuls accumulating in fp32 PSUM; bias is fused into the PSUM->SBUF
eviction on the Scalar engine. Host-side we pre-transpose x (and W) to
put the contraction dim on SBUF partitions, so no on-chip transposes
are needed.

Loop order per core: batch-chunk (bq, 4 x 512 cols) outer, output-tile
(mo, 8 x 128 rows) middle, contraction (ko, 8 x 128) inner. Each
128 KiB x chunk is reused for all 8 mo tiles as soon as it lands, so
compute starts ~1.5 us after the first chunks instead of racing the
full 6 MiB input load; each (bq, mo) PSUM bank is evicted and DMA'd
out while later tiles compute, so there is no output pile-up at the
tail.
"""

import sys

if "/opt/trn_rl_repo" not in sys.path:
    sys.path.insert(0, "/opt/trn_rl_repo")

import ml_dtypes
import numpy as np

BATCH = 16384
IN_F = 1024
OUT_F = 1024
NCORES = 8
P = 128
KO = IN_F // P  # 8 contraction tiles
MO = OUT_F // P  # 8 output-feature tiles
BS = BATCH // NCORES  # 2048 rows per core
FD = 512  # matmul moving free dim (one PSUM bank of fp32)
NB = BS // FD  # 4 moving chunks per core

_cache = {}
LAST_RESULT = None


def _build():
    import concourse.mybir as mybir
    import concourse.tile as tile
    from concourse import bacc

    nc = bacc.Bacc(None, target_bir_lowering=False)
    xT = nc.declare_dram_parameter("xT", [P, KO, BS], mybir.dt.bfloat16, isOutput=False)
    # w2[p, mo, ko, c] = W[mo*P + c, ko*P + p] — per-mo contiguous so the
    # first mo sweep only gates on a 256 KiB chunk.
    w2 = nc.declare_dram_parameter(
        "w2", [P, MO, KO, P], mybir.dt.bfloat16, isOutput=False
    )
    bias = nc.declare_dram_parameter("bias", [P, MO], mybir.dt.float32, isOutput=False)
    out = nc.declare_dram_parameter("out", [P, MO, BS], mybir.dt.float32, isOutput=True)

    with tile.TileContext(nc) as tc:
        with (
            tc.tile_pool(name="const", bufs=1) as cpool,
            tc.tile_pool(name="outp", bufs=4) as opool,
            tc.tile_pool(name="psum", bufs=6, space="PSUM") as ppool,
        ):
            x_sb = cpool.tile([P, KO, BS], mybir.dt.bfloat16)
            w_sb = cpool.tile([P, MO, KO, P], mybir.dt.bfloat16)
            b_sb = cpool.tile([P, MO], mybir.dt.float32)
            nc.sync.dma_start(b_sb[:], bias[:])
            # DMA issue order matches consumption order: w[mo=0], then the
            # x chunks for bq=0, then the remaining w, then remaining x.
            nc.sync.dma_start(w_sb[:, 0], w2[:, 0])
            for ko in range(KO):
                nc.sync.dma_start(x_sb[:, ko, 0:FD], xT[:, ko, 0:FD])
            for mo in range(1, MO):
                nc.sync.dma_start(w_sb[:, mo], w2[:, mo])
            for bq in range(1, NB):
                for ko in range(KO):
                    nc.sync.dma_start(
                        x_sb[:, ko, bq * FD : (bq + 1) * FD],
                        xT[:, ko, bq * FD : (bq + 1) * FD],
                    )

            for bq in range(NB):
                bsl = slice(bq * FD, (bq + 1) * FD)
                for mo in range(MO):
                    ps = ppool.tile([P, FD], mybir.dt.float32)
                    for ko in range(KO):
                        nc.tensor.matmul(
                            ps[:],
                            w_sb[:, mo, ko],
                            x_sb[:, ko, bsl],
                            start=(ko == 0),
                            stop=(ko == KO - 1),
                        )
                    o_sb = opool.tile([P, FD], mybir.dt.float32)
                    nc.scalar.activation(
                        o_sb[:],
                        ps[:],
                        mybir.ActivationFunctionType.Identity,
                        bias=b_sb[:, mo : mo + 1],
                    )
                    nc.sync.dma_start(out[:, mo, bsl], o_sb[:])

    nc.compile()
    return nc


def kernel(x, weight, bias):
    global LAST_RESULT
    from concourse.bass_utils import run_bass_kernel_spmd

    if "nc" not in _cache:
        _cache["nc"] = _build()
    nc = _cache["nc"]

    bf16 = ml_dtypes.bfloat16
    # w2[p, mo, ko, c] = W[mo*P + c, ko*P + p]
    wb = weight.astype(bf16).reshape(MO, P, KO, P)  # [mo, c, ko, p]
    w2 = np.ascontiguousarray(wb.transpose(3, 0, 2, 1))  # [p, mo, ko, c]
    # bias laid out [P, MO]: b[p, mo] = bias[mo*P + p]
    b_t = np.ascontiguousarray(bias.astype(np.float32).reshape(MO, P).T)

    in_maps = []
    for c in range(NCORES):
        xs = x[c * BS : (c + 1) * BS].astype(bf16)
        # x.T laid out [P, KO, BS]: xT[p, ko, b] = x[b, ko*P + p]
        xT = np.ascontiguousarray(xs.T.reshape(KO, P, BS).transpose(1, 0, 2))
        in_maps.append({"xT": xT, "w2": w2, "bias": b_t})

    res = run_bass_kernel_spmd(nc, in_maps, list(range(NCORES)))
    LAST_RESULT = res

    y = np.empty((BATCH, OUT_F), dtype=np.float32)
    for c in range(NCORES):
        o = res.results[c]["out"]  # [P, MO, BS]
        y[c * BS : (c + 1) * BS] = o.transpose(2, 1, 0).reshape(BS, OUT_F)
    return y
